# revision 1
# baseline (speedup 1.0000x reference)
"""Trainium2 Bass kernel: packed-varlen causal GQA attention block.

Sharding: tensor-parallel across heads on 8 NeuronCores.
  core c: q-heads [4c, 4c+4), kv-head c.
  Phase 1: QKV projection (bf16 matmuls, fp32 accum) + RoPE -> qT/kT [d, tok], v [tok, d].
  Phase 2: flash-style attention in transposed layout: ST = K^T-tile @ Q  -> exp -> pT,
           attT = V^T-contract(pT), denominators via ones-matmul, normalize.
  Phase 3: AllGather of attT (bf16) across cores, out[:, c*512:(c+1)*512] = att @ wo_cols.
Host only slices/casts/permutes inputs and concatenates the 8 output column slices.
"""

import sys

import numpy as np
import ml_dtypes

if "/opt/trn_rl_repo" not in sys.path:
    sys.path.insert(0, "/opt/trn_rl_repo")

BF16 = ml_dtypes.bfloat16

# Static problem config (matches the reference).
LENS = [1024, 896, 768, 512]
T = 3200
B = 4
DIM, NH, NKV, HD = 4096, 32, 8, 128
THETA = 500000.0
SCALE = 1.0 / float(np.sqrt(HD))
NCORES = 8
QH = NH // NCORES          # 4 q heads per core
QW = QH * HD               # 512 q/att feature cols per core
KC = DIM // 128            # 32 contraction chunks
SEQ_STARTS = [0, 1024, 1920, 2688]
NEG = -30000.0             # additive mask value; exp() underflows to 0

_CACHE = {}


def _build_program():
    import concourse.bass as bass
    import concourse.mybir as mybir
    import concourse.tile as tile
    from concourse import bacc

    f32 = mybir.dt.float32
    bf16 = mybir.dt.bfloat16

    nc = bacc.Bacc("TRN2", target_bir_lowering=False, debug=False,
                   enable_asserts=False, num_devices=NCORES)

    # ---- I/O ----
    xT_d = nc.dram_tensor("xT", [DIM, T], bf16, kind="ExternalInput")
    wq_d = nc.dram_tensor("wq", [128, KC, QW], bf16, kind="ExternalInput")
    wk_d = nc.dram_tensor("wk", [128, KC, HD], bf16, kind="ExternalInput")
    wv_d = nc.dram_tensor("wv", [128, KC, HD], bf16, kind="ExternalInput")
    wo_d = nc.dram_tensor("wo", [128, KC, QW], bf16, kind="ExternalInput")
    cos_d = nc.dram_tensor("cost", [64, T], f32, kind="ExternalInput")
    sin_d = nc.dram_tensor("sint", [64, T], f32, kind="ExternalInput")
    tri_d = nc.dram_tensor("tri", [128, 128], f32, kind="ExternalInput")
    out_d = nc.dram_tensor("out", [T, QW], f32, kind="ExternalOutput")

    CHUNK = 256  # phase-1 token chunk

    with tile.TileContext(nc) as tc:
        with (
            tc.tile_pool(name="sb", bufs=1) as sb,
            tc.tile_pool(name="ps", bufs=2, space="PSUM") as ps,
            tc.tile_pool(name="dram", bufs=1, space="DRAM") as dpool,
        ):
            # ---- resident SBUF tensors ----
            wq_sb = sb.tile([128, KC, QW], bf16, tag="bigw")
            nc.sync.dma_start(wq_sb[:], wq_d.ap())
            wk_sb = sb.tile([128, KC, HD], bf16)
            nc.sync.dma_start(wk_sb[:], wk_d.ap())
            wv_sb = sb.tile([128, KC, HD], bf16)
            nc.sync.dma_start(wv_sb[:], wv_d.ap())
            cos_sb = sb.tile([64, T], f32)
            nc.sync.dma_start(cos_sb[:], cos_d.ap())
            sin_sb = sb.tile([64, T], f32)
            nc.sync.dma_start(sin_sb[:], sin_d.ap())
            tri_sb = sb.tile([128, 128], f32)
            nc.sync.dma_start(tri_sb[:], tri_d.ap())
            ones_sb = sb.tile([128, 128], bf16)
            nc.vector.memset(ones_sb[:], 1.0)

            qT_sb = sb.tile([128, QH, T], bf16)   # per q-head [d, tok], roped, scaled
            kT_sb = sb.tile([128, T], bf16)       # kv head   [d, tok], roped
            v_sb = sb.tile([128, T], bf16)        # [tok-part, d] per 128-token tile

            xT_r = xT_d.ap().rearrange("(a p) t -> p a t", p=128)

            def rope(dst0, dst1, psum, t0, w):
                """dst0/dst1: [64, w] bf16 slices; psum [128, w] f32 (q or k chunk)."""
                p0 = psum[0:64, :]
                p1 = psum[64:128, :]
                cw = cos_sb[:, t0:t0 + w]
                sw = sin_sb[:, t0:t0 + w]
                m0 = sb.tile([64, CHUNK], f32, tag="rtmp", bufs=6)
                nc.vector.tensor_mul(m0[:, :w], p0, cw)
                m1 = sb.tile([64, CHUNK], f32, tag="rtmp", bufs=6)
                nc.vector.tensor_mul(m1[:, :w], p1, sw)
                nc.vector.tensor_sub(dst0, m0[:, :w], m1[:, :w])
                m2 = sb.tile([64, CHUNK], f32, tag="rtmp", bufs=6)
                nc.vector.tensor_mul(m2[:, :w], p0, sw)
                m3 = sb.tile([64, CHUNK], f32, tag="rtmp", bufs=6)
                nc.vector.tensor_mul(m3[:, :w], p1, cw)
                nc.vector.tensor_add(dst1, m2[:, :w], m3[:, :w])

            # ================= Phase 1: QKV + RoPE =================
            for t0 in range(0, T, CHUNK):
                w = min(CHUNK, T - t0)
                xt = sb.tile([128, KC, CHUNK], bf16, tag="xt", bufs=2)
                nc.sync.dma_start(xt[:, :, :w], xT_r[:, :, t0:t0 + w])

                for h in range(QH):
                    qp = ps.tile([128, 512], f32, tag="A", bufs=3)
                    for kc in range(KC):
                        nc.tensor.matmul(
                            qp[:, :w],
                            wq_sb[:, kc, h * HD:(h + 1) * HD],
                            xt[:, kc, :w],
                            start=(kc == 0), stop=(kc == KC - 1),
                        )
                    rope(qT_sb[0:64, h, t0:t0 + w], qT_sb[64:128, h, t0:t0 + w],
                         qp[:, :w], t0, w)

                kp = ps.tile([128, 512], f32, tag="A", bufs=3)
                for kc in range(KC):
                    nc.tensor.matmul(kp[:, :w], wk_sb[:, kc, :], xt[:, kc, :w],
                                     start=(kc == 0), stop=(kc == KC - 1))
                rope(kT_sb[0:64, t0:t0 + w], kT_sb[64:128, t0:t0 + w],
                     kp[:, :w], t0, w)

                for s in range(w // 128):
                    vp = ps.tile([128, 128], f32, tag="B", bufs=2)
                    for kc in range(KC):
                        nc.tensor.matmul(vp[:], xt[:, kc, s * 128:(s + 1) * 128],
                                         wv_sb[:, kc, :],
                                         start=(kc == 0), stop=(kc == KC - 1))
                    nc.any.tensor_copy(v_sb[:, t0 + s * 128:t0 + (s + 1) * 128], vp[:])

            # ================= Phase 2: attention =================
            ag_in = dpool.tile([QW, T], bf16)

            for h in range(QH):
                for b in range(B):
                    s0 = SEQ_STARTS[b]
                    L = LENS[b]
                    for q0 in range(0, L, 512):
                        w = min(512, L - q0)
                        nkt = (q0 + w) // 128
                        pts = []
                        for kb in range(nkt):
                            k0 = kb * 128
                            pt = sb.tile([128, 512], bf16, tag="pT", bufs=10)
                            if k0 + 128 <= q0:
                                st = ps.tile([128, 512], f32, tag="A", bufs=3)
                                nc.tensor.matmul(
                                    st[:, :w], kT_sb[:, s0 + k0:s0 + k0 + 128],
                                    qT_sb[:, h, s0 + q0:s0 + q0 + w],
                                    start=True, stop=True)
                                nc.scalar.activation(
                                    pt[:, :w], st[:, :w],
                                    mybir.ActivationFunctionType.Exp)
                            else:
                                off = k0 - q0
                                wd = w - off
                                st = ps.tile([128, 512], f32, tag="A", bufs=3)
                                nc.tensor.matmul(
                                    st[:, :wd], kT_sb[:, s0 + k0:s0 + k0 + 128],
                                    qT_sb[:, h, s0 + k0:s0 + k0 + wd],
                                    start=True, stop=True)
                                wm = min(128, wd)
                                nc.vector.tensor_add(st[:, :wm], st[:, :wm],
                                                     tri_sb[:, :wm])
                                if off > 0:
                                    nc.vector.memset(pt[:, :off], 0.0)
                                nc.scalar.activation(
                                    pt[:, off:off + wd], st[:, :wd],
                                    mybir.ActivationFunctionType.Exp)
                            pts.append(pt)

                        att = ps.tile([128, 512], f32, tag="B", bufs=2)
                        den = ps.tile([128, 512], f32, tag="C", bufs=2)
                        for j in range(nkt):
                            fl = dict(start=(j == 0), stop=(j == nkt - 1))
                            nc.tensor.matmul(att[:, :w],
                                             v_sb[:, s0 + j * 128:s0 + (j + 1) * 128],
                                             pts[j][:, :w], **fl)
                            nc.tensor.matmul(den[:, :w], ones_sb[:], pts[j][:, :w],
                                             **fl)
                        rec = sb.tile([128, 512], f32, tag="rec", bufs=2)
                        nc.vector.reciprocal(rec[:, :w], den[:, :w])
                        ao = sb.tile([128, 512], bf16, tag="ao", bufs=3)
                        nc.vector.tensor_mul(ao[:, :w], att[:, :w], rec[:, :w])
                        nc.sync.dma_start(
                            ag_in[h * HD:(h + 1) * HD, s0 + q0:s0 + q0 + w],
                            ao[:, :w])

            # ================= Phase 3: AllGather + wo =================
            ag_out = dpool.tile([NH * HD, T], bf16, addr_space="Shared")
            nc.gpsimd.collective_compute(
                "AllGather",
                mybir.AluOpType.bypass,
                replica_groups=[list(range(NCORES))],
                ins=[ag_in.opt()],
                outs=[ag_out.opt()],
            )
            ag_r = ag_out.rearrange("(a p) t -> p a t", p=128)

            wo_sb = sb.tile([128, KC, QW], bf16, tag="bigw")
            nc.sync.dma_start(wo_sb[:], wo_d.ap())

            for tt in range(T // 128):
                aw = sb.tile([128, KC, 128], bf16, tag="aw", bufs=3)
                nc.sync.dma_start(aw[:], ag_r[:, :, tt * 128:(tt + 1) * 128])
                op = ps.tile([128, 512], f32, tag="A", bufs=3)
                for kc in range(KC):
                    nc.tensor.matmul(op[:], aw[:, kc, :], wo_sb[:, kc, :],
                                     start=(kc == 0), stop=(kc == KC - 1))
                os_ = sb.tile([128, 512], f32, tag="os", bufs=2)
                nc.any.tensor_copy(os_[:], op[:])
                nc.sync.dma_start(out_d.ap()[tt * 128:(tt + 1) * 128, :], os_[:])

    nc.compile()
    return nc


def _host_prep(x, wq, wk, wv, wo, positions):
    """Per-core input maps: slice per head group, permute rope pairs, cast bf16."""
    # rope pair permutation within each head: evens then odds
    perm = np.concatenate([np.arange(0, HD, 2), np.arange(1, HD, 2)])

    inv_freq = 1.0 / (THETA ** (np.arange(64, dtype=np.float64) * 2.0 / HD))
    ang = positions.astype(np.float64)[None, :] * inv_freq[:, None]  # [64, T]
    cos_t = np.ascontiguousarray(np.cos(ang).astype(np.float32))
    sin_t = np.ascontiguousarray(np.sin(ang).astype(np.float32))

    tri = np.where(np.arange(128)[None, :] >= np.arange(128)[:, None],
                   np.float32(0.0), np.float32(NEG)).astype(np.float32)

    xT = np.ascontiguousarray(x.T.astype(BF16))

    def shard_w(w_full, cols, permute):
        ws = w_full[:, cols].astype(np.float64)
        if permute is not None:
            nh = ws.shape[1] // HD
            ws = ws.reshape(DIM, nh, HD)[:, :, permute].reshape(DIM, nh * HD)
        return ws

    in_maps = []
    for c in range(NCORES):
        qcols = slice(c * QW, (c + 1) * QW)
        kcols = slice(c * HD, (c + 1) * HD)
        wq_c = shard_w(wq, qcols, perm) * SCALE
        wk_c = shard_w(wk, kcols, perm)
        wv_c = wv[:, kcols].astype(np.float64)
        wo_c = wo[:, qcols].astype(np.float64)

        def lay(wm):  # [DIM, n] -> [128, KC, n] with dim = a*128+p
            n = wm.shape[1]
            return np.ascontiguousarray(
                wm.reshape(KC, 128, n).transpose(1, 0, 2).astype(BF16))

        in_maps.append({
            "xT": xT,
            "wq": lay(wq_c),
            "wk": lay(wk_c),
            "wv": lay(wv_c),
            "wo": lay(wo_c),
            "cost": cos_t,
            "sint": sin_t,
            "tri": tri,
        })
    return in_maps


def _get_program():
    if "nc" not in _CACHE:
        _CACHE["nc"] = _build_program()
    return _CACHE["nc"]


def kernel(x, wq, wk, wv, wo, positions, _trace=False):
    from concourse import bass_utils

    nc = _get_program()
    in_maps = _host_prep(np.asarray(x), np.asarray(wq), np.asarray(wk),
                         np.asarray(wv), np.asarray(wo), np.asarray(positions))
    res = bass_utils.run_bass_kernel_spmd(
        nc, in_maps, core_ids=list(range(NCORES)), trace=_trace)
    _CACHE["last_result"] = res
    out = np.concatenate([res.results[c]["out"] for c in range(NCORES)], axis=1)
    return np.ascontiguousarray(out.astype(np.float32))


# revision 20
# speedup vs baseline: 1.1798x; 1.1798x over previous
"""Trainium2 Bass kernel: packed-varlen causal GQA attention block.

Sharding: tensor-parallel across heads on 8 NeuronCores.
  core c: q-heads [4c, 4c+4), kv-head c.
  Phase 1: QKV projection (bf16 matmuls, fp32 accum) + RoPE -> qT/kT [d, tok], v [tok, d].
  Phase 2: flash-style attention in transposed layout: ST = K-tile^T stationary vs Q
           moving -> exp -> pT; attT = V-contract(pT); denominators via ones-matmul.
  Phase 3: AllGather of attT (bf16) across cores, out[:, c*512:(c+1)*512] = att @ wo_cols.
Host only slices/casts/permutes inputs and concatenates the 8 output column slices.
"""

import sys

import numpy as np
import ml_dtypes

if "/opt/trn_rl_repo" not in sys.path:
    sys.path.insert(0, "/opt/trn_rl_repo")

BF16 = ml_dtypes.bfloat16

# Static problem config (matches the reference).
LENS = [1024, 896, 768, 512]
T = 3200
B = 4
DIM, NH, NKV, HD = 4096, 32, 8, 128
THETA = 500000.0
SCALE = 1.0 / float(np.sqrt(HD))
NCORES = 8
QH = NH // NCORES          # 4 q heads per core
QW = QH * HD               # 512 q/att feature cols per core
KC = DIM // 128            # 32 contraction chunks
SEQ_STARTS = [0, 1024, 1920, 2688]
NEG = -30000.0             # additive mask value; exp() underflows to 0

_CACHE = {}


def _build_program(phases=(1, 2, 3), collective=True, repeat=1, variant=()):
    import concourse.mybir as mybir
    import concourse.tile as tile
    from concourse import bacc

    f32 = mybir.dt.float32
    bf16 = mybir.dt.bfloat16

    nc = bacc.Bacc("TRN2", target_bir_lowering=False, debug=False,
                   enable_asserts=False, num_devices=NCORES)

    # ---- I/O ----
    xT_d = nc.dram_tensor("xT", [DIM, T], bf16, kind="ExternalInput")
    wq_d = nc.dram_tensor("wq", [128, KC, QW], bf16, kind="ExternalInput")
    wk_d = nc.dram_tensor("wk", [128, KC, HD], bf16, kind="ExternalInput")
    wv_d = nc.dram_tensor("wv", [128, KC, HD], bf16, kind="ExternalInput")
    wo_d = nc.dram_tensor("wo", [128, KC, QW], bf16, kind="ExternalInput")
    cos_d = nc.dram_tensor("cost", [64, T], f32, kind="ExternalInput")
    sin_d = nc.dram_tensor("sint", [64, T], f32, kind="ExternalInput")
    tri_d = nc.dram_tensor("tri", [128, 128], f32, kind="ExternalInput")
    out_d = nc.dram_tensor("out", [T, QW], f32, kind="ExternalOutput")

    CHUNK = 256  # phase-1 token chunk

    with tile.TileContext(nc) as tc:
        with (
            tc.tile_pool(name="sb", bufs=1) as sb,
            tc.tile_pool(name="ps", bufs=2, space="PSUM") as ps,
            tc.tile_pool(name="dram", bufs=1, space="DRAM") as dpool,
        ):
            # ---- resident SBUF tensors (shared across reps) ----
            wk_sb = sb.tile([128, KC, HD], bf16)
            nc.sync.dma_start(wk_sb[:], wk_d.ap())
            wv_sb = sb.tile([128, KC, HD], bf16)
            nc.sync.dma_start(wv_sb[:], wv_d.ap())
            cos_sb = sb.tile([64, T], f32)
            nc.sync.dma_start(cos_sb[:], cos_d.ap())
            sin_sb = sb.tile([64, T], f32)
            nc.sync.dma_start(sin_sb[:], sin_d.ap())
            tri_sb = sb.tile([128, 128], f32)
            nc.sync.dma_start(tri_sb[:], tri_d.ap())
            ones_sb = sb.tile([128, 128], bf16)
            nc.vector.memset(ones_sb[:], 1.0)

            qT_sb = sb.tile([128, QH, T], bf16)   # per q-head [d, tok], roped+scaled
            kT_sb = sb.tile([128, T], bf16)       # kv head   [d, tok], roped
            v_sb = sb.tile([128, T], bf16)        # [tok-part, d] per 128-token tile

            xT_r = xT_d.ap().rearrange("(a p) t -> p a t", p=128)

            def rope(dst0, dst1, psum, t0, w):
                """dst0/dst1: [64, w] bf16 slices; psum [128, w] f32 (q or k chunk)."""
                p0 = psum[0:64, :]
                p1 = psum[64:128, :]
                cw = cos_sb[:, t0:t0 + w]
                sw = sin_sb[:, t0:t0 + w]
                m0 = sb.tile([64, CHUNK], f32, tag="rtmp", bufs=4)
                nc.vector.tensor_mul(m0[:, :w], p0, cw)
                m1 = sb.tile([64, CHUNK], f32, tag="rtmp", bufs=4)
                nc.vector.tensor_mul(m1[:, :w], p1, sw)
                nc.vector.tensor_sub(dst0, m0[:, :w], m1[:, :w])
                m2 = sb.tile([64, CHUNK], f32, tag="rtmp", bufs=4)
                nc.vector.tensor_mul(m2[:, :w], p0, sw)
                m3 = sb.tile([64, CHUNK], f32, tag="rtmp", bufs=4)
                nc.vector.tensor_mul(m3[:, :w], p1, cw)
                nc.vector.tensor_add(dst1, m2[:, :w], m3[:, :w])

            for _rep in range(repeat):
                # ---- Phase 1: QKV + RoPE ----
                if 1 in phases:
                    # wq/wo share one SBUF slot (tag bigw); per-rep alloc keeps
                    # the slot rotation consistent with program order.
                    wq_sb = sb.tile([128, KC, QW], bf16, tag="bigw")
                    nc.sync.dma_start(wq_sb[:], wq_d.ap())
                for t0 in range(0, T, CHUNK) if 1 in phases else []:
                    w = min(CHUNK, T - t0)
                    xt = sb.tile([128, KC, CHUNK], bf16, tag="xt", bufs=2)
                    nc.sync.dma_start(xt[:, :, :w], xT_r[:, :, t0:t0 + w])

                    for h in range(QH):
                        qp = ps.tile([128, 512], f32, tag="A", bufs=3)
                        for kc in range(KC):
                            nc.tensor.matmul(
                                qp[:, :w],
                                wq_sb[:, kc, h * HD:(h + 1) * HD],
                                xt[:, kc, :w],
                                start=(kc == 0), stop=(kc == KC - 1),
                            )
                        rope(qT_sb[0:64, h, t0:t0 + w],
                             qT_sb[64:128, h, t0:t0 + w], qp[:, :w], t0, w)

                    kp = ps.tile([128, 512], f32, tag="A", bufs=3)
                    for kc in range(KC):
                        nc.tensor.matmul(kp[:, :w], wk_sb[:, kc, :], xt[:, kc, :w],
                                         start=(kc == 0), stop=(kc == KC - 1))
                    rope(kT_sb[0:64, t0:t0 + w], kT_sb[64:128, t0:t0 + w],
                         kp[:, :w], t0, w)

                    for s in range(w // 128):
                        vp = ps.tile([128, 128], f32, tag="B", bufs=2)
                        for kc in range(KC):
                            nc.tensor.matmul(vp[:], xt[:, kc, s * 128:(s + 1) * 128],
                                             wv_sb[:, kc, :],
                                             start=(kc == 0), stop=(kc == KC - 1))
                        nc.any.tensor_copy(
                            v_sb[:, t0 + s * 128:t0 + (s + 1) * 128], vp[:])

                # ---- Phase 2: attention (seq-outer so per-seq AG can overlap) ----
                ag_ins = [dpool.tile([QW, LENS[b]], bf16, tag=f"agin{b}",
                                     name=f"agin{b}")
                          for b in range(B)] if 2 in phases or 3 in phases else []

                for b in range(B) if 2 in phases else []:
                    s0 = SEQ_STARTS[b]
                    L = LENS[b]
                    ag_in = ag_ins[b]
                    for h in range(QH):
                        for q0 in range(0, L, 512):
                            w = min(512, L - q0)
                            nkt = (q0 + w) // 128
                            pts = []
                            ptbufs = 16 if "bufs" in variant else 10
                            stbufs = 4 if "bufs" in variant else 3
                            for kb in range(nkt):
                                k0 = kb * 128
                                pt = sb.tile([128, 512], bf16, tag="pT",
                                             bufs=ptbufs)
                                if k0 + 128 <= q0:
                                    st = ps.tile([128, 512], f32, tag="A",
                                                 bufs=stbufs)
                                    nc.tensor.matmul(
                                        st[:, :w], kT_sb[:, s0 + k0:s0 + k0 + 128],
                                        qT_sb[:, h, s0 + q0:s0 + q0 + w],
                                        start=True, stop=True)
                                    if "nexp" in variant:
                                        nc.vector.tensor_copy(pt[:, :w], st[:, :w])
                                    else:
                                        nc.scalar.activation(
                                            pt[:, :w], st[:, :w],
                                            mybir.ActivationFunctionType.Exp)
                                else:
                                    off = k0 - q0
                                    wd = w - off
                                    st = ps.tile([128, 512], f32, tag="A",
                                                 bufs=stbufs)
                                    nc.tensor.matmul(
                                        st[:, :wd], kT_sb[:, s0 + k0:s0 + k0 + 128],
                                        qT_sb[:, h, s0 + k0:s0 + k0 + wd],
                                        start=True, stop=True)
                                    wm = min(128, wd)
                                    if "notri" not in variant:
                                        nc.vector.tensor_add(st[:, :wm], st[:, :wm],
                                                             tri_sb[:, :wm])
                                    if off > 0:
                                        nc.vector.memset(pt[:, :off], 0.0)
                                    if "nexp" in variant:
                                        nc.vector.tensor_copy(pt[:, off:off + wd],
                                                              st[:, :wd])
                                    else:
                                        nc.scalar.activation(
                                            pt[:, off:off + wd], st[:, :wd],
                                            mybir.ActivationFunctionType.Exp)
                                pts.append(pt)

                            att = ps.tile([128, 512], f32, tag="B", bufs=2)
                            if "noden" not in variant:
                                den = ps.tile([128, 512], f32, tag="C", bufs=2)
                            interleave = "ildgrp" not in variant
                            for j in range(nkt):
                                fl = dict(start=(j == 0), stop=(j == nkt - 1))
                                nc.tensor.matmul(
                                    att[:, :w],
                                    v_sb[:, s0 + j * 128:s0 + (j + 1) * 128],
                                    pts[j][:, :w], **fl)
                                if "noden" not in variant and interleave:
                                    nc.tensor.matmul(den[:, :w], ones_sb[:],
                                                     pts[j][:, :w], **fl)
                            if "noden" not in variant and not interleave:
                                for j in range(nkt):
                                    fl = dict(start=(j == 0), stop=(j == nkt - 1))
                                    nc.tensor.matmul(den[:, :w], ones_sb[:],
                                                     pts[j][:, :w], **fl)
                            ao = sb.tile([128, 512], bf16, tag="ao", bufs=2)
                            if "noden" in variant or "norec" in variant:
                                nc.vector.tensor_copy(ao[:, :w], att[:, :w])
                            else:
                                rec = sb.tile([128, 512], f32, tag="rec", bufs=2)
                                if "slowrec" in variant:
                                    nc.vector.reciprocal(rec[:, :w], den[:, :w])
                                else:
                                    # DVE exact reciprocal is an 8-deep iterative
                                    # divide (~8x slower); 18-bit approx is plenty
                                    # for a softmax denom feeding bf16.
                                    nc.vector.reciprocal_approx_fast(rec[:, :w],
                                                                     den[:, :w])
                                nc.vector.tensor_mul(ao[:, :w], att[:, :w],
                                                     rec[:, :w])
                            nc.sync.dma_start(
                                ag_in[h * HD:(h + 1) * HD, q0:q0 + w],
                                ao[:, :w])

                    # ---- Phase 3 for this sequence: AllGather + wo ----
                    if 3 in phases:
                        if b == 0:
                            # overlaps attention; slot shared with wq (tag bigw)
                            wo_sb = sb.tile([128, KC, QW], bf16, tag="bigw",
                                            name="wo_sb")
                            nc.sync.dma_start(wo_sb[:], wo_d.ap())
                        ag_out = dpool.tile(
                            [NH * HD, L], bf16, tag=f"agout{b}", name=f"agout{b}",
                            addr_space="Shared" if collective else "Local")
                        if collective:
                            nc.gpsimd.collective_compute(
                                "AllGather",
                                mybir.AluOpType.bypass,
                                replica_groups=[list(range(NCORES))],
                                ins=[ag_in.opt()],
                                outs=[ag_out.opt()],
                            )
                        else:  # single-core sim stand-in: replicate 8x
                            for r in range(NCORES):
                                nc.sync.dma_start(
                                    ag_out[r * QW:(r + 1) * QW, :], ag_in[:])
                        ag_r = ag_out.rearrange("(a p) t -> p a t", p=128)

                        for t0 in range(0, L, 256):
                            wl = min(256, L - t0)
                            aw = sb.tile([128, KC, 256], bf16, tag="aw", bufs=2)
                            nc.sync.dma_start(aw[:, :, :wl],
                                              ag_r[:, :, t0:t0 + wl])
                            for s in range(wl // 128):
                                op = ps.tile([128, 512], f32, tag="A", bufs=3)
                                for kc in range(KC):
                                    nc.tensor.matmul(
                                        op[:], aw[:, kc, s * 128:(s + 1) * 128],
                                        wo_sb[:, kc, :],
                                        start=(kc == 0), stop=(kc == KC - 1))
                                os_ = sb.tile([128, 512], f32, tag="os", bufs=2)
                                nc.any.tensor_copy(os_[:], op[:])
                                nc.sync.dma_start(
                                    out_d.ap()[s0 + t0 + s * 128:
                                               s0 + t0 + (s + 1) * 128, :],
                                    os_[:])

    nc.compile()
    return nc


def _host_prep(x, wq, wk, wv, wo, positions):
    """Per-core input maps: slice per head group, permute rope pairs, cast bf16."""
    # rope pair permutation within each head: evens then odds
    perm = np.concatenate([np.arange(0, HD, 2), np.arange(1, HD, 2)])

    inv_freq = 1.0 / (THETA ** (np.arange(64, dtype=np.float64) * 2.0 / HD))
    ang = positions.astype(np.float64)[None, :] * inv_freq[:, None]  # [64, T]
    cos_t = np.ascontiguousarray(np.cos(ang).astype(np.float32))
    sin_t = np.ascontiguousarray(np.sin(ang).astype(np.float32))

    tri = np.where(np.arange(128)[None, :] >= np.arange(128)[:, None],
                   np.float32(0.0), np.float32(NEG)).astype(np.float32)

    xT = np.ascontiguousarray(x.T.astype(BF16))

    def shard_w(w_full, cols, permute):
        ws = w_full[:, cols].astype(np.float64)
        if permute is not None:
            nh = ws.shape[1] // HD
            ws = ws.reshape(DIM, nh, HD)[:, :, permute].reshape(DIM, nh * HD)
        return ws

    in_maps = []
    for c in range(NCORES):
        qcols = slice(c * QW, (c + 1) * QW)
        kcols = slice(c * HD, (c + 1) * HD)
        wq_c = shard_w(wq, qcols, perm) * SCALE
        wk_c = shard_w(wk, kcols, perm)
        wv_c = wv[:, kcols].astype(np.float64)
        wo_c = wo[:, qcols].astype(np.float64)

        def lay(wm):  # [DIM, n] -> [128, KC, n] with dim = a*128+p
            n = wm.shape[1]
            return np.ascontiguousarray(
                wm.reshape(KC, 128, n).transpose(1, 0, 2).astype(BF16))

        in_maps.append({
            "xT": xT,
            "wq": lay(wq_c),
            "wk": lay(wk_c),
            "wv": lay(wv_c),
            "wo": lay(wo_c),
            "cost": cos_t,
            "sint": sin_t,
            "tri": tri,
        })
    return in_maps


def _get_program():
    if "nc" not in _CACHE:
        _CACHE["nc"] = _build_program()
    return _CACHE["nc"]


def kernel(x, wq, wk, wv, wo, positions, _trace=False):
    from concourse import bass_utils

    nc = _get_program()
    in_maps = _host_prep(np.asarray(x), np.asarray(wq), np.asarray(wk),
                         np.asarray(wv), np.asarray(wo), np.asarray(positions))
    res = bass_utils.run_bass_kernel_spmd(
        nc, in_maps, core_ids=list(range(NCORES)), trace=_trace)
    _CACHE["last_result"] = res
    out = np.concatenate([res.results[c]["out"] for c in range(NCORES)], axis=1)
    return np.ascontiguousarray(out.astype(np.float32))


# revision 21
# speedup vs baseline: 16250.8536x; 13774.0479x over previous
"""Trainium2 Bass kernel: packed-varlen causal GQA attention block.

Sharding: tensor-parallel across heads on 8 NeuronCores.
  core c: q-heads [4c, 4c+4), kv-head c.
  Phase 1: QKV projection (bf16 matmuls, fp32 accum) + RoPE -> qT/kT [d, tok], v [tok, d].
  Phase 2: flash-style attention in transposed layout: ST = K-tile^T stationary vs Q
           moving -> exp -> pT; attT = V-contract(pT); denominators via ones-matmul.
  Phase 3: AllGather of attT (bf16) across cores, out[:, c*512:(c+1)*512] = att @ wo_cols.
Host only slices/casts/permutes inputs and concatenates the 8 output column slices.
"""

import sys

import numpy as np
import ml_dtypes

if "/opt/trn_rl_repo" not in sys.path:
    sys.path.insert(0, "/opt/trn_rl_repo")

BF16 = ml_dtypes.bfloat16

# Static problem config (matches the reference).
LENS = [1024, 896, 768, 512]
T = 3200
B = 4
DIM, NH, NKV, HD = 4096, 32, 8, 128
THETA = 500000.0
SCALE = 1.0 / float(np.sqrt(HD))
NCORES = 8
QH = NH // NCORES          # 4 q heads per core
QW = QH * HD               # 512 q/att feature cols per core
KC = DIM // 128            # 32 contraction chunks
SEQ_STARTS = [0, 1024, 1920, 2688]
NEG = -30000.0             # additive mask value; exp() underflows to 0

_CACHE = {}


def _build_program(phases=(1, 2, 3), collective=True, repeat=1, variant=()):
    import concourse.mybir as mybir
    import concourse.tile as tile
    from concourse import bacc

    f32 = mybir.dt.float32
    bf16 = mybir.dt.bfloat16

    nc = bacc.Bacc("TRN2", target_bir_lowering=False, debug=False,
                   enable_asserts=False, num_devices=NCORES)

    # ---- I/O ----
    xT_d = nc.dram_tensor("xT", [DIM, T], bf16, kind="ExternalInput")
    wq_d = nc.dram_tensor("wq", [128, KC, QW], bf16, kind="ExternalInput")
    wk_d = nc.dram_tensor("wk", [128, KC, HD], bf16, kind="ExternalInput")
    wv_d = nc.dram_tensor("wv", [128, KC, HD], bf16, kind="ExternalInput")
    wo_d = nc.dram_tensor("wo", [128, KC, QW], bf16, kind="ExternalInput")
    cos_d = nc.dram_tensor("cost", [64, T], f32, kind="ExternalInput")
    sin_d = nc.dram_tensor("sint", [64, T], f32, kind="ExternalInput")
    tri_d = nc.dram_tensor("tri", [128, 128], f32, kind="ExternalInput")
    out_d = nc.dram_tensor("out", [T, QW], f32, kind="ExternalOutput")

    CHUNK = 256  # phase-1 token chunk

    with tile.TileContext(nc) as tc:
        with (
            tc.tile_pool(name="sb", bufs=1) as sb,
            tc.tile_pool(name="ps", bufs=2, space="PSUM") as ps,
            tc.tile_pool(name="dram", bufs=1, space="DRAM") as dpool,
        ):
            # ---- resident SBUF tensors (shared across reps) ----
            wk_sb = sb.tile([128, KC, HD], bf16)
            nc.sync.dma_start(wk_sb[:], wk_d.ap())
            wv_sb = sb.tile([128, KC, HD], bf16)
            nc.sync.dma_start(wv_sb[:], wv_d.ap())
            cos_sb = sb.tile([64, T], f32)
            nc.sync.dma_start(cos_sb[:], cos_d.ap())
            sin_sb = sb.tile([64, T], f32)
            nc.sync.dma_start(sin_sb[:], sin_d.ap())
            tri_sb = sb.tile([128, 128], f32)
            nc.sync.dma_start(tri_sb[:], tri_d.ap())
            ones_sb = sb.tile([128, 128], bf16)
            nc.vector.memset(ones_sb[:], 1.0)

            qT_sb = sb.tile([128, QH, T], bf16)   # per q-head [d, tok], roped+scaled
            kT_sb = sb.tile([128, T], bf16)       # kv head   [d, tok], roped
            v_sb = sb.tile([128, T], bf16)        # [tok-part, d] per 128-token tile

            xT_r = xT_d.ap().rearrange("(a p) t -> p a t", p=128)

            def rope(dst0, dst1, psum, t0, w):
                """dst0/dst1: [64, w] bf16 slices; psum [128, w] f32 (q or k chunk)."""
                p0 = psum[0:64, :]
                p1 = psum[64:128, :]
                cw = cos_sb[:, t0:t0 + w]
                sw = sin_sb[:, t0:t0 + w]
                m0 = sb.tile([64, CHUNK], f32, tag="rtmp", bufs=4)
                nc.vector.tensor_mul(m0[:, :w], p0, cw)
                m1 = sb.tile([64, CHUNK], f32, tag="rtmp", bufs=4)
                nc.vector.tensor_mul(m1[:, :w], p1, sw)
                nc.vector.tensor_sub(dst0, m0[:, :w], m1[:, :w])
                m2 = sb.tile([64, CHUNK], f32, tag="rtmp", bufs=4)
                nc.vector.tensor_mul(m2[:, :w], p0, sw)
                m3 = sb.tile([64, CHUNK], f32, tag="rtmp", bufs=4)
                nc.vector.tensor_mul(m3[:, :w], p1, cw)
                nc.vector.tensor_add(dst1, m2[:, :w], m3[:, :w])

            for _rep in range(repeat):
                # ---- Phase 1: QKV + RoPE ----
                if 1 in phases:
                    # wq/wo share one SBUF slot (tag bigw); per-rep alloc keeps
                    # the slot rotation consistent with program order.
                    wq_sb = sb.tile([128, KC, QW], bf16, tag="bigw")
                    nc.sync.dma_start(wq_sb[:], wq_d.ap())
                for t0 in range(0, T, CHUNK) if 1 in phases else []:
                    w = min(CHUNK, T - t0)
                    xt = sb.tile([128, KC, CHUNK], bf16, tag="xt", bufs=2)
                    nc.sync.dma_start(xt[:, :, :w], xT_r[:, :, t0:t0 + w])

                    for h in range(QH):
                        qp = ps.tile([128, 512], f32, tag="A", bufs=2)
                        for kc in range(KC):
                            nc.tensor.matmul(
                                qp[:, :w],
                                wq_sb[:, kc, h * HD:(h + 1) * HD],
                                xt[:, kc, :w],
                                start=(kc == 0), stop=(kc == KC - 1),
                            )
                        rope(qT_sb[0:64, h, t0:t0 + w],
                             qT_sb[64:128, h, t0:t0 + w], qp[:, :w], t0, w)

                    kp = ps.tile([128, 512], f32, tag="A", bufs=2)
                    for kc in range(KC):
                        nc.tensor.matmul(kp[:, :w], wk_sb[:, kc, :], xt[:, kc, :w],
                                         start=(kc == 0), stop=(kc == KC - 1))
                    rope(kT_sb[0:64, t0:t0 + w], kT_sb[64:128, t0:t0 + w],
                         kp[:, :w], t0, w)

                    for s in range(w // 128):
                        vp = ps.tile([128, 128], f32, tag="B", bufs=2)
                        for kc in range(KC):
                            nc.tensor.matmul(vp[:], xt[:, kc, s * 128:(s + 1) * 128],
                                             wv_sb[:, kc, :],
                                             start=(kc == 0), stop=(kc == KC - 1))
                        nc.any.tensor_copy(
                            v_sb[:, t0 + s * 128:t0 + (s + 1) * 128], vp[:])

                # ---- Phase 2: attention (seq-outer so per-seq AG can overlap) ----
                ag_ins = [dpool.tile([QW, LENS[b]], bf16, tag=f"agin{b}",
                                     name=f"agin{b}")
                          for b in range(B)] if 2 in phases or 3 in phases else []

                for b in range(B) if 2 in phases else []:
                    s0 = SEQ_STARTS[b]
                    L = LENS[b]
                    ag_in = ag_ins[b]
                    for h in range(QH):
                        for q0 in range(0, L, 512):
                            w = min(512, L - q0)
                            nkt = (q0 + w) // 128
                            pts = []
                            ptbufs = 16 if "bufs" in variant else 10
                            stbufs = 4 if "bufs" in variant else 2
                            for kb in range(nkt):
                                k0 = kb * 128
                                pt = sb.tile([128, 512], bf16, tag="pT",
                                             bufs=ptbufs)
                                if k0 + 128 <= q0:
                                    st = ps.tile([128, 512], f32, tag="A",
                                                 bufs=stbufs)
                                    nc.tensor.matmul(
                                        st[:, :w], kT_sb[:, s0 + k0:s0 + k0 + 128],
                                        qT_sb[:, h, s0 + q0:s0 + q0 + w],
                                        start=True, stop=True)
                                    if "nexp" in variant:
                                        nc.vector.tensor_copy(pt[:, :w], st[:, :w])
                                    else:
                                        nc.scalar.activation(
                                            pt[:, :w], st[:, :w],
                                            mybir.ActivationFunctionType.Exp)
                                else:
                                    off = k0 - q0
                                    wd = w - off
                                    st = ps.tile([128, 512], f32, tag="A",
                                                 bufs=stbufs)
                                    nc.tensor.matmul(
                                        st[:, :wd], kT_sb[:, s0 + k0:s0 + k0 + 128],
                                        qT_sb[:, h, s0 + k0:s0 + k0 + wd],
                                        start=True, stop=True)
                                    wm = min(128, wd)
                                    if "notri" not in variant:
                                        nc.vector.tensor_add(st[:, :wm], st[:, :wm],
                                                             tri_sb[:, :wm])
                                    if off > 0:
                                        nc.vector.memset(pt[:, :off], 0.0)
                                    if "nexp" in variant:
                                        nc.vector.tensor_copy(pt[:, off:off + wd],
                                                              st[:, :wd])
                                    else:
                                        nc.scalar.activation(
                                            pt[:, off:off + wd], st[:, :wd],
                                            mybir.ActivationFunctionType.Exp)
                                pts.append(pt)

                            att = ps.tile([128, 512], f32, tag="B", bufs=2)
                            if "noden" not in variant:
                                den = ps.tile([128, 512], f32, tag="C", bufs=2)
                            interleave = "ildgrp" not in variant
                            for j in range(nkt):
                                fl = dict(start=(j == 0), stop=(j == nkt - 1))
                                nc.tensor.matmul(
                                    att[:, :w],
                                    v_sb[:, s0 + j * 128:s0 + (j + 1) * 128],
                                    pts[j][:, :w], **fl)
                                if "noden" not in variant and interleave:
                                    nc.tensor.matmul(den[:, :w], ones_sb[:],
                                                     pts[j][:, :w], **fl)
                            if "noden" not in variant and not interleave:
                                for j in range(nkt):
                                    fl = dict(start=(j == 0), stop=(j == nkt - 1))
                                    nc.tensor.matmul(den[:, :w], ones_sb[:],
                                                     pts[j][:, :w], **fl)
                            ao = sb.tile([128, 512], bf16, tag="ao", bufs=2)
                            if "noden" in variant or "norec" in variant:
                                nc.vector.tensor_copy(ao[:, :w], att[:, :w])
                            else:
                                rec = sb.tile([128, 512], f32, tag="rec", bufs=2)
                                if "slowrec" in variant:
                                    nc.vector.reciprocal(rec[:, :w], den[:, :w])
                                else:
                                    # DVE exact reciprocal is an 8-deep iterative
                                    # divide (~8x slower); 18-bit approx is plenty
                                    # for a softmax denom feeding bf16.
                                    nc.vector.reciprocal_approx_fast(rec[:, :w],
                                                                     den[:, :w])
                                nc.vector.tensor_mul(ao[:, :w], att[:, :w],
                                                     rec[:, :w])
                            nc.sync.dma_start(
                                ag_in[h * HD:(h + 1) * HD, q0:q0 + w],
                                ao[:, :w])

                    # ---- Phase 3 for this sequence: AllGather + wo ----
                    if 3 in phases:
                        if b == 0:
                            # overlaps attention; slot shared with wq (tag bigw)
                            wo_sb = sb.tile([128, KC, QW], bf16, tag="bigw",
                                            name="wo_sb")
                            nc.sync.dma_start(wo_sb[:], wo_d.ap())
                        ag_out = dpool.tile(
                            [NH * HD, L], bf16, tag=f"agout{b}", name=f"agout{b}",
                            addr_space="Shared" if collective else "Local")
                        if collective:
                            nc.gpsimd.collective_compute(
                                "AllGather",
                                mybir.AluOpType.bypass,
                                replica_groups=[list(range(NCORES))],
                                ins=[ag_in.opt()],
                                outs=[ag_out.opt()],
                            )
                        else:  # single-core sim stand-in: replicate 8x
                            for r in range(NCORES):
                                nc.sync.dma_start(
                                    ag_out[r * QW:(r + 1) * QW, :], ag_in[:])
                        ag_r = ag_out.rearrange("(a p) t -> p a t", p=128)

                        for t0 in range(0, L, 256):
                            wl = min(256, L - t0)
                            aw = sb.tile([128, KC, 256], bf16, tag="aw", bufs=2)
                            nc.sync.dma_start(aw[:, :, :wl],
                                              ag_r[:, :, t0:t0 + wl])
                            for s in range(wl // 128):
                                op = ps.tile([128, 512], f32, tag="D", bufs=2)
                                for kc in range(KC):
                                    nc.tensor.matmul(
                                        op[:], aw[:, kc, s * 128:(s + 1) * 128],
                                        wo_sb[:, kc, :],
                                        start=(kc == 0), stop=(kc == KC - 1))
                                os_ = sb.tile([128, 512], f32, tag="os", bufs=2)
                                nc.any.tensor_copy(os_[:], op[:])
                                nc.sync.dma_start(
                                    out_d.ap()[s0 + t0 + s * 128:
                                               s0 + t0 + (s + 1) * 128, :],
                                    os_[:])

    nc.compile()
    return nc


def _host_prep(x, wq, wk, wv, wo, positions):
    """Per-core input maps: slice per head group, permute rope pairs, cast bf16."""
    # rope pair permutation within each head: evens then odds
    perm = np.concatenate([np.arange(0, HD, 2), np.arange(1, HD, 2)])

    inv_freq = 1.0 / (THETA ** (np.arange(64, dtype=np.float64) * 2.0 / HD))
    ang = positions.astype(np.float64)[None, :] * inv_freq[:, None]  # [64, T]
    cos_t = np.ascontiguousarray(np.cos(ang).astype(np.float32))
    sin_t = np.ascontiguousarray(np.sin(ang).astype(np.float32))

    tri = np.where(np.arange(128)[None, :] >= np.arange(128)[:, None],
                   np.float32(0.0), np.float32(NEG)).astype(np.float32)

    xT = np.ascontiguousarray(x.T.astype(BF16))

    def shard_w(w_full, cols, permute):
        ws = w_full[:, cols].astype(np.float64)
        if permute is not None:
            nh = ws.shape[1] // HD
            ws = ws.reshape(DIM, nh, HD)[:, :, permute].reshape(DIM, nh * HD)
        return ws

    in_maps = []
    for c in range(NCORES):
        qcols = slice(c * QW, (c + 1) * QW)
        kcols = slice(c * HD, (c + 1) * HD)
        wq_c = shard_w(wq, qcols, perm) * SCALE
        wk_c = shard_w(wk, kcols, perm)
        wv_c = wv[:, kcols].astype(np.float64)
        wo_c = wo[:, qcols].astype(np.float64)

        def lay(wm):  # [DIM, n] -> [128, KC, n] with dim = a*128+p
            n = wm.shape[1]
            return np.ascontiguousarray(
                wm.reshape(KC, 128, n).transpose(1, 0, 2).astype(BF16))

        in_maps.append({
            "xT": xT,
            "wq": lay(wq_c),
            "wk": lay(wk_c),
            "wv": lay(wv_c),
            "wo": lay(wo_c),
            "cost": cos_t,
            "sint": sin_t,
            "tri": tri,
        })
    return in_maps


def _get_program():
    if "nc" not in _CACHE:
        _CACHE["nc"] = _build_program()
    return _CACHE["nc"]


def kernel(x, wq, wk, wv, wo, positions, _trace=False):
    from concourse import bass_utils

    nc = _get_program()
    in_maps = _host_prep(np.asarray(x), np.asarray(wq), np.asarray(wk),
                         np.asarray(wv), np.asarray(wo), np.asarray(positions))
    res = bass_utils.run_bass_kernel_spmd(
        nc, in_maps, core_ids=list(range(NCORES)), trace=_trace)
    _CACHE["last_result"] = res
    out = np.concatenate([res.results[c]["out"] for c in range(NCORES)], axis=1)
    return np.ascontiguousarray(out.astype(np.float32))
